# revision 1
# baseline (speedup 1.0000x reference)
"""Trainium2 Bass kernel for CircularUpsample2 (upfirdn2d up=2, circular pad).

out[b,c] = A @ x[b,c] @ B^T  per image, where A,B are (256,128) banded
circulant polyphase-upsample matrices built host-side from the 4x4 FIR
kernel (separable; the reference kernel is exactly rank-1 with bf16-exact
taps).

Device strategy (per core, pure data parallel over the 2048 b*c images):
  MM1: s = x^T A^T      (lhsT = x,  rhs = A^T)  -> PSUM (w, 2H)
  MM2: out_blk = s_blk^T B^T  for 2 row blocks  -> PSUM (rows, 2W)
No transposes needed anywhere. Fast path runs the matmuls in bf16 with
hi/lo mantissa splitting (x split on host; s split on device), which is
4x faster on the PE than fp32 and accurate to ~4e-6 relative.
"""

import numpy as np
import ml_dtypes

import concourse.bass as bass
from concourse import bacc
import concourse.mybir as mybir
from concourse.tile import TileContext
from concourse.bass_utils import run_bass_kernel_spmd

BF16 = ml_dtypes.bfloat16
N_CORES = 8
H = W = 128
OH = OW = 256


# ---------------------------------------------------------------- host math
def _build_M(taps, n=H):
    """1-D polyphase factor (2n, n):
    out[2t]   = taps[2]*x[(t-2)%n] + taps[0]*x[(t-1)%n]
    out[2t+1] = taps[3]*x[(t-2)%n] + taps[1]*x[(t-1)%n]
    """
    M = np.zeros((2 * n, n), dtype=np.float32)
    t = np.arange(n)
    M[2 * t, (t - 2) % n] += taps[2]
    M[2 * t, (t - 1) % n] += taps[0]
    M[2 * t + 1, (t - 2) % n] += taps[3]
    M[2 * t + 1, (t - 1) % n] += taps[1]
    return M


def _factorize(k):
    """k (4,4) float32 -> list of (u, v) float32 with k = sum_r outer(u,v).

    Prefers an exact symmetric factorization for rank-1 PSD kernels so the
    taps stay exactly representable (the reference kernel's taps are
    0.25/0.75, exact in bf16).
    """
    k64 = k.astype(np.float64)
    U, S, Vt = np.linalg.svd(k64)
    rank = int(np.sum(S > 1e-7 * S[0]))
    if rank == 1:
        i = int(np.argmax(np.abs(np.diag(k64))))
        if k64[i, i] > 0:
            r = np.sqrt(k64[i, i])
            u = (k64[i, :] / r).astype(np.float32)
            if np.allclose(np.outer(u, u), k64, rtol=1e-6, atol=1e-9):
                return [(u, u.copy())]
        u = (U[:, 0] * S[0]).astype(np.float32)
        v = Vt[0, :].astype(np.float32)
        return [(u, v)]
    return [((U[:, r] * S[r]).astype(np.float32), Vt[r, :].astype(np.float32))
            for r in range(rank)]


# ---------------------------------------------------------------- bass build
def _build_nc_bf16(n_img, g_load=32, g_store=4):
    """bf16 hi/lo split path: A,B must be bf16-exact, rank 1.

    Inputs xh/xl host-transposed to (H, n_img, W) for contiguous loads;
    A row-permuted to [even; odd] so each partition's store chunk is the
    2KB-contiguous row pair (2p, 2p+1). Two images per PSUM tile; the PE
    stream is software-pipelined one pair ahead (MM1 of pair i+1 is
    emitted before MM2 of pair i) so MM2 never stalls on the hi/lo split
    copies.
    """
    assert n_img % g_load == 0 and n_img % g_store == 0 and g_store % 2 == 0
    nc = bacc.Bacc("TRN2", target_bir_lowering=False)
    bf = mybir.dt.bfloat16
    f32 = mybir.dt.float32
    xh_d = nc.dram_tensor("xh", (H, n_img, W), bf, kind="ExternalInput")
    xl_d = nc.dram_tensor("xl", (H, n_img, W), bf, kind="ExternalInput")
    at_d = nc.dram_tensor("at", (H, OH), bf, kind="ExternalInput")
    bt_d = nc.dram_tensor("bt", (W, OW), bf, kind="ExternalInput")
    out_d = nc.dram_tensor("out", (n_img, OH, OW), f32, kind="ExternalOutput")

    with TileContext(nc) as tc:
        with tc.tile_pool(name="consts", bufs=1) as cpool, \
             tc.tile_pool(name="data", bufs=4) as pool, \
             tc.tile_pool(name="psum", bufs=1, space="PSUM") as ppool:
            at = cpool.tile([H, OH], bf)
            nc.scalar.dma_start(out=at, in_=at_d[:])
            bt = cpool.tile([W, OW], bf)
            nc.scalar.dma_start(out=bt, in_=bt_d[:])

            osb_tiles = {}

            sizes = [8, 8, 16] if n_img >= 64 else []
            rem = n_img - sum(sizes)
            sizes += [g_load] * (rem // g_load) + ([rem % g_load] if rem % g_load else [])
            load_groups = {}
            b0 = 0
            for sz in sizes:
                load_groups[b0] = sz
                b0 += sz

            def stage1(p0):
                """loads + MM1 + hi/lo split for image pair (p0, p0+1)."""
                if p0 in load_groups:
                    g0, gsz = p0, load_groups[p0]
                    xhg = pool.tile([128, gsz * W], bf, tag="xhg", bufs=3,
                                    name=f"xhg_{g0}")
                    nc.sync.dma_start(out=xhg.rearrange("p (g w) -> p g w", g=gsz),
                                      in_=xh_d[:, g0:g0 + gsz, :])
                    xlg = pool.tile([128, gsz * W], bf, tag="xlg", bufs=3,
                                    name=f"xlg_{g0}")
                    nc.sync.dma_start(out=xlg.rearrange("p (g w) -> p g w", g=gsz),
                                      in_=xl_d[:, g0:g0 + gsz, :])
                    stage1.xhg, stage1.xlg, stage1.g0 = xhg, xlg, g0
                xhg, xlg, g0 = stage1.xhg, stage1.xlg, stage1.g0
                s2_p = ppool.tile([128, 2 * OH], f32, tag="s2_p", bufs=2,
                                  name=f"s2_p_{p0}")
                for q in range(2):
                    gi = p0 + q - g0
                    nc.tensor.matmul(s2_p[:, q * OH:(q + 1) * OH],
                                     lhsT=xhg[:, gi * W:(gi + 1) * W],
                                     rhs=at, start=True, stop=False)
                    nc.tensor.matmul(s2_p[:, q * OH:(q + 1) * OH],
                                     lhsT=xlg[:, gi * W:(gi + 1) * W],
                                     rhs=at, start=False, stop=True)
                s_hi = pool.tile([128, 2 * OH], bf, tag="s_hi", bufs=3,
                                 name=f"s_hi_{p0}")
                nc.scalar.copy(out=s_hi, in_=s2_p)
                s_lo = pool.tile([128, 2 * OH], bf, tag="s_lo", bufs=3,
                                 name=f"s_lo_{p0}")
                nc.vector.tensor_sub(s_lo, s2_p, s_hi)
                return s_hi, s_lo

            def stage2(p0, s_hi, s_lo):
                """MM2 + output copy + (on group tail) the store DMA."""
                s0 = (p0 // g_store) * g_store
                if s0 not in osb_tiles:
                    osb_tiles[s0] = pool.tile([128, g_store * 2 * OW], f32,
                                              tag="o_sb", bufs=4, name=f"o_sb_{s0}")
                o_sb = osb_tiles[s0]
                o2_p = ppool.tile([128, 4 * OW], f32, tag="o2_p", bufs=3,
                                  name=f"o2_p_{p0}")
                for q in range(2):
                    for r in range(2):
                        dst = o2_p[:, (q * 2 + r) * OW:(q * 2 + r + 1) * OW]
                        lo = q * OH + r * 128
                        nc.tensor.matmul(dst, lhsT=s_hi[:, lo:lo + 128],
                                         rhs=bt, start=True, stop=False)
                        nc.tensor.matmul(dst, lhsT=s_lo[:, lo:lo + 128],
                                         rhs=bt, start=False, stop=True)
                oq = (p0 - s0) * 2 * OW
                dst = o_sb[:, oq:oq + 4 * OW]
                if (p0 // 2) % 2 == 0:
                    nc.vector.tensor_copy(dst, o2_p)
                else:
                    nc.scalar.copy(out=dst, in_=o2_p)
                last = s0 + g_store == n_img
                if last:
                    hg = g_store // 2
                    done = p0 + 2 - s0
                    for h0 in (0, hg):
                        if done == h0 + hg:
                            nc.sync.dma_start(
                                out=out_d[s0 + h0:s0 + h0 + hg].rearrange(
                                    "g (p r) j -> p g r j", r=2),
                                in_=o_sb[:, h0 * 2 * OW:(h0 + hg) * 2 * OW]
                                    .rearrange("p (g r j) -> p g r j",
                                               g=hg, r=2))
                elif p0 + 2 == s0 + g_store:
                    nc.sync.dma_start(
                        out=out_d[s0:s0 + g_store].rearrange(
                            "g (p r) j -> p g r j", r=2),
                        in_=o_sb.rearrange("p (g r j) -> p g r j",
                                           g=g_store, r=2))
                    del osb_tiles[s0]

            prev = None
            for p0 in range(0, n_img, 2):
                cur = (p0, *stage1(p0))
                if prev is not None:
                    stage2(*prev)
                prev = cur
            stage2(*prev)
    nc.finalize()
    return nc


def _build_nc_fp32(n_img, n_terms, g_load=8, g_store=2):
    """general fp32 path, rank n_terms."""
    nc = bacc.Bacc("TRN2", target_bir_lowering=False)
    f32 = mybir.dt.float32
    R = n_terms
    x_d = nc.dram_tensor("x", (n_img, H, W), f32, kind="ExternalInput")
    at_d = nc.dram_tensor("at", (R, H, OH), f32, kind="ExternalInput")
    bt_d = nc.dram_tensor("bt", (R, W, OW), f32, kind="ExternalInput")
    out_d = nc.dram_tensor("out", (n_img, OH, OW), f32, kind="ExternalOutput")

    with TileContext(nc) as tc:
        with tc.tile_pool(name="consts", bufs=1) as cpool, \
             tc.tile_pool(name="data", bufs=4) as pool, \
             tc.tile_pool(name="psum", bufs=4, space="PSUM") as ppool:
            at = cpool.tile([H, R * OH], f32)
            nc.sync.dma_start(out=at.rearrange("p (r i) -> p r i", r=R), in_=at_d.rearrange("r h i -> h r i"))
            bt = cpool.tile([W, R * OW], f32)
            nc.sync.dma_start(out=bt.rearrange("p (r j) -> p r j", r=R), in_=bt_d.rearrange("r w j -> w r j"))

            for s0 in range(0, n_img, g_store):
                o_sb = pool.tile([128, g_store * 2 * OW], f32, tag="o_sb")
                for img in range(s0, s0 + g_store):
                    gi = img % g_load
                    if gi == 0:
                        g0 = img
                        xg = pool.tile([128, g_load * W], f32, tag="xg")
                        nc.sync.dma_start(
                            out=xg.rearrange("p (g w) -> p g w", g=g_load),
                            in_=x_d[g0:g0 + g_load].rearrange("g h w -> h g w"))
                    x_t = xg[:, gi * W:(gi + 1) * W]

                    s_p = ppool.tile([128, R * OH], f32, tag="s_p")
                    for r in range(R):
                        nc.tensor.matmul(s_p[:, r * OH:(r + 1) * OH], lhsT=x_t,
                                         rhs=at[:, r * OH:(r + 1) * OH],
                                         start=True, stop=True)
                    s_sb = pool.tile([128, R * OH], f32, tag="s_sb")
                    nc.scalar.copy(out=s_sb, in_=s_p)

                    oq = (img - s0) * 2 * OW
                    for blk in range(2):
                        o_p = ppool.tile([128, OW], f32, tag="o_p")
                        for r in range(R):
                            nc.tensor.matmul(
                                o_p,
                                lhsT=s_sb[:, r * OH + blk * 128: r * OH + (blk + 1) * 128],
                                rhs=bt[:, r * OW:(r + 1) * OW],
                                start=(r == 0), stop=(r == R - 1))
                        dst = o_sb[:, oq + blk * OW: oq + (blk + 1) * OW]
                        if blk == 0:
                            nc.vector.tensor_copy(dst, o_p)
                        else:
                            nc.scalar.copy(out=dst, in_=o_p)
                nc.sync.dma_start(
                    out=out_d[s0:s0 + g_store].rearrange("g (b p) j -> p g b j", b=2),
                    in_=o_sb.rearrange("p (g b j) -> p g b j", g=g_store, b=2))
    nc.finalize()
    return nc


_NC_CACHE = {}


def _get_nc(key, builder):
    if key not in _NC_CACHE:
        _NC_CACHE[key] = builder()
    return _NC_CACHE[key]


# ---------------------------------------------------------------- entry
def _run(x, kern, trace=False, n_cores=N_CORES):
    xf = np.ascontiguousarray(np.asarray(x, dtype=np.float32))
    k = np.asarray(kern, dtype=np.float32)
    b, c, h, w = xf.shape
    assert (h, w) == (H, W), (h, w)
    n_tot = b * c
    assert n_tot % n_cores == 0
    n_per = n_tot // n_cores
    imgs = xf.reshape(n_tot, h, w)

    terms = _factorize(k)
    fast = None
    if len(terms) == 1:
        A = _build_M(terms[0][0])
        Bm = _build_M(terms[0][1])
        if (np.array_equal(A.astype(BF16).astype(np.float32), A)
                and np.array_equal(Bm.astype(BF16).astype(np.float32), Bm)):
            fast = (A.astype(BF16), Bm.astype(BF16))

    if fast is not None:
        Ab, Bb = fast
        nc = _get_nc(("bf16", n_per), lambda: _build_nc_bf16(n_per))
        xh = imgs.astype(BF16)
        xl = (imgs - xh.astype(np.float32)).astype(BF16)
        # permute A rows to [even; odd] so MM2 block r produces rows 2p+r
        Ap = np.concatenate([Ab[0::2], Ab[1::2]], axis=0)
        at = np.ascontiguousarray(Ap.T)
        bt = np.ascontiguousarray(Bb.T)
        in_maps = [
            {"xh": np.ascontiguousarray(
                 xh[i * n_per:(i + 1) * n_per].transpose(1, 0, 2)),
             "xl": np.ascontiguousarray(
                 xl[i * n_per:(i + 1) * n_per].transpose(1, 0, 2)),
             "at": at, "bt": bt}
            for i in range(n_cores)
        ]
    else:
        R = len(terms)
        nc = _get_nc(("fp32", n_per, R), lambda: _build_nc_fp32(n_per, R))
        at = np.ascontiguousarray(
            np.stack([_build_M(u).T for (u, v) in terms]))
        bt = np.ascontiguousarray(
            np.stack([_build_M(v).T for (u, v) in terms]))
        in_maps = [
            {"x": imgs[i * n_per:(i + 1) * n_per], "at": at, "bt": bt}
            for i in range(n_cores)
        ]

    res = run_bass_kernel_spmd(nc, in_maps, list(range(n_cores)), trace=trace)
    out = np.concatenate([res.results[i]["out"] for i in range(n_cores)], axis=0)
    return out.reshape(b, c, OH, OW), res


def kernel(x, kernel):
    out, _ = _run(x, kernel, trace=False)
    return out



# revision 4
# speedup vs baseline: 1.7546x; 1.7546x over previous
"""Trainium2 Bass kernel for CircularUpsample2 (upfirdn2d up=2, circular pad).

out[b,c] = A @ x[b,c] @ B^T  per image, where A,B are (256,128) banded
circulant polyphase-upsample matrices built host-side from the 4x4 FIR
kernel (separable; the reference kernel is exactly rank-1).

Device strategy (per core, pure data parallel over the 2048 b*c images):
  MM1: s = x^T A^T      (lhsT = x,  rhs = A^T)  -> PSUM (w, 2H)
  MM2: out_blk = s_blk^T B^T  for 2 row blocks  -> PSUM (rows, 2W)
No transposes needed anywhere. Fast path runs everything in fp16
(x, A^T, B^T, s, out all fp16; PSUM accumulates fp32): fp16 matmuls
stream at the same rate as bf16 on the PE but with 10 mantissa bits,
giving ~2e-4 relative error with half the PE work and half the store
traffic of the previous bf16 hi/lo + fp32-out version. The fp16 output
is upcast to fp32 on the host.
"""

import numpy as np

import concourse.bass as bass
from concourse import bacc
import concourse.mybir as mybir
from concourse.tile import TileContext
from concourse.bass_utils import run_bass_kernel_spmd

N_CORES = 8
H = W = 128
OH = OW = 256


# ---------------------------------------------------------------- host math
def _build_M(taps, n=H):
    """1-D polyphase factor (2n, n):
    out[2t]   = taps[2]*x[(t-2)%n] + taps[0]*x[(t-1)%n]
    out[2t+1] = taps[3]*x[(t-2)%n] + taps[1]*x[(t-1)%n]
    """
    M = np.zeros((2 * n, n), dtype=np.float32)
    t = np.arange(n)
    M[2 * t, (t - 2) % n] += taps[2]
    M[2 * t, (t - 1) % n] += taps[0]
    M[2 * t + 1, (t - 2) % n] += taps[3]
    M[2 * t + 1, (t - 1) % n] += taps[1]
    return M


def _factorize(k):
    """k (4,4) float32 -> list of (u, v) float32 with k = sum_r outer(u,v)."""
    k64 = k.astype(np.float64)
    U, S, Vt = np.linalg.svd(k64)
    rank = int(np.sum(S > 1e-7 * S[0]))
    if rank == 1:
        i = int(np.argmax(np.abs(np.diag(k64))))
        if k64[i, i] > 0:
            r = np.sqrt(k64[i, i])
            u = (k64[i, :] / r).astype(np.float32)
            if np.allclose(np.outer(u, u), k64, rtol=1e-6, atol=1e-9):
                return [(u, u.copy())]
        u = (U[:, 0] * S[0]).astype(np.float32)
        v = Vt[0, :].astype(np.float32)
        return [(u, v)]
    return [((U[:, r] * S[r]).astype(np.float32), Vt[r, :].astype(np.float32))
            for r in range(rank)]


# ---------------------------------------------------------------- bass build
def _build_nc_f16(n_img, g_load=32, g_store=4):
    """fp16 single-pass path (rank-1 kernels).

    x host-transposed to (H, n_img, W) for contiguous loads; A row-permuted
    to [even; odd] so each partition's store chunk is the contiguous row
    pair (2p, 2p+1). Two images per PSUM tile; the PE stream is software-
    pipelined one pair ahead (MM1 of pair i+1 is emitted before MM2 of
    pair i) so MM2 never stalls on the s copy. PSUM->SBUF fp16 downconvert
    copies rotate over scalar (s) and vector/gpsimd (out).
    """
    assert n_img % g_load == 0 and n_img % g_store == 0 and g_store % 2 == 0
    nc = bacc.Bacc("TRN2", target_bir_lowering=False)
    f16 = mybir.dt.float16
    f32 = mybir.dt.float32
    x_d = nc.dram_tensor("x", (H, n_img, W), f16, kind="ExternalInput")
    at_d = nc.dram_tensor("at", (H, OH), f16, kind="ExternalInput")
    bt_d = nc.dram_tensor("bt", (W, OW), f16, kind="ExternalInput")
    out_d = nc.dram_tensor("out", (n_img, OH, OW), f16, kind="ExternalOutput")

    with TileContext(nc) as tc:
        with tc.tile_pool(name="consts", bufs=1) as cpool, \
             tc.tile_pool(name="data", bufs=4) as pool, \
             tc.tile_pool(name="psum", bufs=1, space="PSUM") as ppool:
            at = cpool.tile([H, OH], f16)
            nc.scalar.dma_start(out=at, in_=at_d[:])
            bt = cpool.tile([W, OW], f16)
            nc.scalar.dma_start(out=bt, in_=bt_d[:])

            osb_tiles = {}

            sizes = [8, 8, 16] if n_img >= 64 else []
            rem = n_img - sum(sizes)
            sizes += [g_load] * (rem // g_load) + ([rem % g_load] if rem % g_load else [])
            load_groups = {}
            b0 = 0
            for sz in sizes:
                load_groups[b0] = sz
                b0 += sz

            def stage1(p0):
                """loads + MM1 + s downconvert for image pair (p0, p0+1)."""
                if p0 in load_groups:
                    g0, gsz = p0, load_groups[p0]
                    xg = pool.tile([128, gsz * W], f16, tag="xg", bufs=3,
                                   name=f"xg_{g0}")
                    nc.sync.dma_start(out=xg.rearrange("p (g w) -> p g w", g=gsz),
                                      in_=x_d[:, g0:g0 + gsz, :])
                    stage1.xg, stage1.g0 = xg, g0
                xg, g0 = stage1.xg, stage1.g0
                s2_p = ppool.tile([128, 2 * OH], f32, tag="s2_p", bufs=2,
                                  name=f"s2_p_{p0}")
                for q in range(2):
                    gi = p0 + q - g0
                    nc.tensor.matmul(s2_p[:, q * OH:(q + 1) * OH],
                                     lhsT=xg[:, gi * W:(gi + 1) * W],
                                     rhs=at, start=True, stop=True)
                s_sb = pool.tile([128, 2 * OH], f16, tag="s_sb", bufs=3,
                                 name=f"s_sb_{p0}")
                if (p0 // 2) % 2 == 0:
                    nc.scalar.copy(out=s_sb, in_=s2_p)
                else:
                    nc.vector.tensor_copy(s_sb, s2_p)
                return s_sb

            def stage2(p0, s_sb):
                """MM2 + output downconvert + (on group tail) the store DMA."""
                s0 = (p0 // g_store) * g_store
                if s0 not in osb_tiles:
                    osb_tiles[s0] = pool.tile([128, g_store * 2 * OW], f16,
                                              tag="o_sb", bufs=4, name=f"o_sb_{s0}")
                o_sb = osb_tiles[s0]
                o2_p = ppool.tile([128, 4 * OW], f32, tag="o2_p", bufs=3,
                                  name=f"o2_p_{p0}")
                for q in range(2):
                    for r in range(2):
                        dst = o2_p[:, (q * 2 + r) * OW:(q * 2 + r + 1) * OW]
                        lo = q * OH + r * 128
                        nc.tensor.matmul(dst, lhsT=s_sb[:, lo:lo + 128],
                                         rhs=bt, start=True, stop=True)
                oq = (p0 - s0) * 2 * OW
                dst = o_sb[:, oq:oq + 4 * OW]
                if (p0 // 2) % 2 == 0:
                    nc.vector.tensor_copy(dst, o2_p)
                else:
                    nc.scalar.copy(out=dst, in_=o2_p)
                last = s0 + g_store == n_img
                if last:
                    hg = g_store // 2
                    done = p0 + 2 - s0
                    for h0 in (0, hg):
                        if done == h0 + hg:
                            nc.sync.dma_start(
                                out=out_d[s0 + h0:s0 + h0 + hg].rearrange(
                                    "g (p r) j -> p g r j", r=2),
                                in_=o_sb[:, h0 * 2 * OW:(h0 + hg) * 2 * OW]
                                    .rearrange("p (g r j) -> p g r j",
                                               g=hg, r=2))
                elif p0 + 2 == s0 + g_store:
                    nc.sync.dma_start(
                        out=out_d[s0:s0 + g_store].rearrange(
                            "g (p r) j -> p g r j", r=2),
                        in_=o_sb.rearrange("p (g r j) -> p g r j",
                                           g=g_store, r=2))
                    del osb_tiles[s0]

            prev = None
            for p0 in range(0, n_img, 2):
                cur = (p0, stage1(p0))
                if prev is not None:
                    stage2(*prev)
                prev = cur
            stage2(*prev)
    nc.finalize()
    return nc


def _build_nc_fp32(n_img, n_terms, g_load=8, g_store=2):
    """general fp32 path, rank n_terms."""
    nc = bacc.Bacc("TRN2", target_bir_lowering=False)
    f32 = mybir.dt.float32
    R = n_terms
    x_d = nc.dram_tensor("x", (n_img, H, W), f32, kind="ExternalInput")
    at_d = nc.dram_tensor("at", (R, H, OH), f32, kind="ExternalInput")
    bt_d = nc.dram_tensor("bt", (R, W, OW), f32, kind="ExternalInput")
    out_d = nc.dram_tensor("out", (n_img, OH, OW), f32, kind="ExternalOutput")

    with TileContext(nc) as tc:
        with tc.tile_pool(name="consts", bufs=1) as cpool, \
             tc.tile_pool(name="data", bufs=4) as pool, \
             tc.tile_pool(name="psum", bufs=4, space="PSUM") as ppool:
            at = cpool.tile([H, R * OH], f32)
            nc.sync.dma_start(out=at.rearrange("p (r i) -> p r i", r=R), in_=at_d.rearrange("r h i -> h r i"))
            bt = cpool.tile([W, R * OW], f32)
            nc.sync.dma_start(out=bt.rearrange("p (r j) -> p r j", r=R), in_=bt_d.rearrange("r w j -> w r j"))

            for s0 in range(0, n_img, g_store):
                o_sb = pool.tile([128, g_store * 2 * OW], f32, tag="o_sb")
                for img in range(s0, s0 + g_store):
                    gi = img % g_load
                    if gi == 0:
                        g0 = img
                        xg = pool.tile([128, g_load * W], f32, tag="xg")
                        nc.sync.dma_start(
                            out=xg.rearrange("p (g w) -> p g w", g=g_load),
                            in_=x_d[g0:g0 + g_load].rearrange("g h w -> h g w"))
                    x_t = xg[:, gi * W:(gi + 1) * W]

                    s_p = ppool.tile([128, R * OH], f32, tag="s_p")
                    for r in range(R):
                        nc.tensor.matmul(s_p[:, r * OH:(r + 1) * OH], lhsT=x_t,
                                         rhs=at[:, r * OH:(r + 1) * OH],
                                         start=True, stop=True)
                    s_sb = pool.tile([128, R * OH], f32, tag="s_sb")
                    nc.scalar.copy(out=s_sb, in_=s_p)

                    oq = (img - s0) * 2 * OW
                    for blk in range(2):
                        o_p = ppool.tile([128, OW], f32, tag="o_p")
                        for r in range(R):
                            nc.tensor.matmul(
                                o_p,
                                lhsT=s_sb[:, r * OH + blk * 128: r * OH + (blk + 1) * 128],
                                rhs=bt[:, r * OW:(r + 1) * OW],
                                start=(r == 0), stop=(r == R - 1))
                        dst = o_sb[:, oq + blk * OW: oq + (blk + 1) * OW]
                        if blk == 0:
                            nc.vector.tensor_copy(dst, o_p)
                        else:
                            nc.scalar.copy(out=dst, in_=o_p)
                nc.sync.dma_start(
                    out=out_d[s0:s0 + g_store].rearrange("g (b p) j -> p g b j", b=2),
                    in_=o_sb.rearrange("p (g b j) -> p g b j", g=g_store, b=2))
    nc.finalize()
    return nc


_NC_CACHE = {}


def _get_nc(key, builder):
    if key not in _NC_CACHE:
        _NC_CACHE[key] = builder()
    return _NC_CACHE[key]


# ---------------------------------------------------------------- entry
def _run(x, kern, trace=False, n_cores=N_CORES):
    xf = np.ascontiguousarray(np.asarray(x, dtype=np.float32))
    k = np.asarray(kern, dtype=np.float32)
    b, c, h, w = xf.shape
    assert (h, w) == (H, W), (h, w)
    n_tot = b * c
    assert n_tot % n_cores == 0
    n_per = n_tot // n_cores
    imgs = xf.reshape(n_tot, h, w)

    terms = _factorize(k)

    if len(terms) == 1:
        A = _build_M(terms[0][0])
        Bm = _build_M(terms[0][1])
        nc = _get_nc(("f16", n_per), lambda: _build_nc_f16(n_per))
        x16 = imgs.astype(np.float16)
        # permute A rows to [even; odd] so MM2 block r produces rows 2p+r
        Ap = np.concatenate([A[0::2], A[1::2]], axis=0)
        at = np.ascontiguousarray(Ap.T.astype(np.float16))
        bt = np.ascontiguousarray(Bm.T.astype(np.float16))
        in_maps = [
            {"x": np.ascontiguousarray(
                 x16[i * n_per:(i + 1) * n_per].transpose(1, 0, 2)),
             "at": at, "bt": bt}
            for i in range(n_cores)
        ]
        res = run_bass_kernel_spmd(nc, in_maps, list(range(n_cores)), trace=trace)
        out = np.concatenate([res.results[i]["out"] for i in range(n_cores)],
                             axis=0).astype(np.float32)
    else:
        R = len(terms)
        nc = _get_nc(("fp32", n_per, R), lambda: _build_nc_fp32(n_per, R))
        at = np.ascontiguousarray(
            np.stack([_build_M(u).T for (u, v) in terms]))
        bt = np.ascontiguousarray(
            np.stack([_build_M(v).T for (u, v) in terms]))
        in_maps = [
            {"x": imgs[i * n_per:(i + 1) * n_per], "at": at, "bt": bt}
            for i in range(n_cores)
        ]
        res = run_bass_kernel_spmd(nc, in_maps, list(range(n_cores)), trace=trace)
        out = np.concatenate([res.results[i]["out"] for i in range(n_cores)], axis=0)
    return out.reshape(b, c, OH, OW), res


def kernel(x, kernel):
    out, _ = _run(x, kernel, trace=False)
    return out


# revision 6
# speedup vs baseline: 2.1209x; 1.2087x over previous
"""Trainium2 Bass kernel for CircularUpsample2 (upfirdn2d up=2, circular pad).

out[b,c] = A @ x[b,c] @ B^T  per image, where A,B are (256,128) banded
circulant polyphase-upsample matrices built host-side from the 4x4 FIR
kernel (separable; the reference kernel is exactly rank-1).

Device strategy (per core, pure data parallel over the 2048 b*c images):
  MM1: s = x^T A^T      (lhsT = x,  rhs = A^T)  -> PSUM (w, 2H)
  MM2: out_blk = s_blk^T B^T  for 2 row blocks  -> PSUM (rows, 2W)
Everything runs in fp16 (PSUM accumulates fp32): fp16 matmuls stream at
the same rate as bf16 on the PE but with 10 mantissa bits, giving ~2e-4
relative error (gate is 2e-2) with half the PE work and half the store
traffic of a bf16 hi/lo + fp32-out scheme.

The output DRAM tensor is laid out partition-major (p, n_img, 2, OW): each
partition's chunk for a store group is fully contiguous in DRAM, so store
descriptors aggregate into 8 KB DMA packets instead of the 1 KB forced by
the natural (n_img, OH, OW) layout (DMA engines sustain ~26 B/ns at >=2 KB
packets). The host re-gathers rows (cheap: fused with the fp32 upcast).
"""

import numpy as np

import concourse.bass as bass
from concourse import bacc
import concourse.mybir as mybir
from concourse.tile import TileContext
from concourse.bass_utils import run_bass_kernel_spmd

N_CORES = 8
H = W = 128
OH = OW = 256


# ---------------------------------------------------------------- host math
def _build_M(taps, n=H):
    """1-D polyphase factor (2n, n):
    out[2t]   = taps[2]*x[(t-2)%n] + taps[0]*x[(t-1)%n]
    out[2t+1] = taps[3]*x[(t-2)%n] + taps[1]*x[(t-1)%n]
    """
    M = np.zeros((2 * n, n), dtype=np.float32)
    t = np.arange(n)
    M[2 * t, (t - 2) % n] += taps[2]
    M[2 * t, (t - 1) % n] += taps[0]
    M[2 * t + 1, (t - 2) % n] += taps[3]
    M[2 * t + 1, (t - 1) % n] += taps[1]
    return M


def _factorize(k):
    """k (4,4) float32 -> list of (u, v) float32 with k = sum_r outer(u,v)."""
    k64 = k.astype(np.float64)
    U, S, Vt = np.linalg.svd(k64)
    rank = int(np.sum(S > 1e-7 * S[0]))
    if rank == 1:
        i = int(np.argmax(np.abs(np.diag(k64))))
        if k64[i, i] > 0:
            r = np.sqrt(k64[i, i])
            u = (k64[i, :] / r).astype(np.float32)
            if np.allclose(np.outer(u, u), k64, rtol=1e-6, atol=1e-9):
                return [(u, u.copy())]
        u = (U[:, 0] * S[0]).astype(np.float32)
        v = Vt[0, :].astype(np.float32)
        return [(u, v)]
    return [((U[:, r] * S[r]).astype(np.float32), Vt[r, :].astype(np.float32))
            for r in range(rank)]


# ---------------------------------------------------------------- bass build
def _build_nc_f16(n_img, g_load=32, g_store=8):
    """fp16 single-pass path (rank-1 kernels).

    x host-transposed to (H, n_img, W) for contiguous loads; A row-permuted
    to [even; odd] so MM2 block r produces output rows 2p+r on partition p.
    Two images per PSUM tile; the PE stream is software-pipelined one pair
    ahead (MM1 of pair i+1 is emitted before MM2 of pair i) so MM2 never
    stalls on the s copy. PSUM->SBUF fp16 downconvert copies alternate
    between scalar and vector.
    """
    assert n_img % g_load == 0 and n_img % g_store == 0 and g_store % 2 == 0
    nc = bacc.Bacc("TRN2", target_bir_lowering=False)
    f16 = mybir.dt.float16
    f32 = mybir.dt.float32
    x_d = nc.dram_tensor("x", (H, n_img, W), f16, kind="ExternalInput")
    at_d = nc.dram_tensor("at", (H, OH), f16, kind="ExternalInput")
    bt_d = nc.dram_tensor("bt", (W, OW), f16, kind="ExternalInput")
    # partition-major: out[p, g, r, :] = row (2p+r) of image g
    out_d = nc.dram_tensor("out", (128, n_img, 2, OW), f16,
                           kind="ExternalOutput")

    with TileContext(nc) as tc:
        with tc.tile_pool(name="consts", bufs=1) as cpool, \
             tc.tile_pool(name="data", bufs=4) as pool, \
             tc.tile_pool(name="psum", bufs=1, space="PSUM") as ppool:
            at = cpool.tile([H, OH], f16)
            nc.scalar.dma_start(out=at, in_=at_d[:])
            bt = cpool.tile([W, OW], f16)
            nc.scalar.dma_start(out=bt, in_=bt_d[:])

            osb_tiles = {}

            sizes = [8, 8, 16] if n_img >= 64 else []
            rem = n_img - sum(sizes)
            sizes += [g_load] * (rem // g_load) + ([rem % g_load] if rem % g_load else [])
            load_groups = {}
            b0 = 0
            for sz in sizes:
                load_groups[b0] = sz
                b0 += sz

            def stage1(p0):
                """loads + MM1 + s downconvert for image pair (p0, p0+1)."""
                if p0 in load_groups:
                    g0, gsz = p0, load_groups[p0]
                    xg = pool.tile([128, gsz * W], f16, tag="xg", bufs=3,
                                   name=f"xg_{g0}")
                    nc.scalar.dma_start(out=xg.rearrange("p (g w) -> p g w", g=gsz),
                                        in_=x_d[:, g0:g0 + gsz, :])
                    stage1.xg, stage1.g0 = xg, g0
                xg, g0 = stage1.xg, stage1.g0
                s2_p = ppool.tile([128, 2 * OH], f32, tag="s2_p", bufs=2,
                                  name=f"s2_p_{p0}")
                for q in range(2):
                    gi = p0 + q - g0
                    nc.tensor.matmul(s2_p[:, q * OH:(q + 1) * OH],
                                     lhsT=xg[:, gi * W:(gi + 1) * W],
                                     rhs=at, start=True, stop=True)
                s_sb = pool.tile([128, 2 * OH], f16, tag="s_sb", bufs=3,
                                 name=f"s_sb_{p0}")
                if (p0 // 2) % 2 == 0:
                    nc.scalar.copy(out=s_sb, in_=s2_p)
                else:
                    nc.vector.tensor_copy(s_sb, s2_p)
                return s_sb

            def stage2(p0, s_sb):
                """MM2 + output downconvert + (on group tail) the store DMA."""
                s0 = (p0 // g_store) * g_store
                if s0 not in osb_tiles:
                    osb_tiles[s0] = pool.tile([128, g_store * 2 * OW], f16,
                                              tag="o_sb", bufs=4, name=f"o_sb_{s0}")
                o_sb = osb_tiles[s0]
                o2_p = ppool.tile([128, 4 * OW], f32, tag="o2_p", bufs=3,
                                  name=f"o2_p_{p0}")
                for q in range(2):
                    for r in range(2):
                        dst = o2_p[:, (q * 2 + r) * OW:(q * 2 + r + 1) * OW]
                        lo = q * OH + r * 128
                        nc.tensor.matmul(dst, lhsT=s_sb[:, lo:lo + 128],
                                         rhs=bt, start=True, stop=True)
                oq = (p0 - s0) * 2 * OW
                dst = o_sb[:, oq:oq + 4 * OW]
                if (p0 // 2) % 2 == 0:
                    nc.vector.tensor_copy(dst, o2_p)
                else:
                    nc.scalar.copy(out=dst, in_=o2_p)
                last = s0 + g_store == n_img
                if last:
                    hg = g_store // 2
                    done = p0 + 2 - s0
                    for h0 in (0, hg):
                        if done == h0 + hg:
                            nc.sync.dma_start(
                                out=out_d[:, s0 + h0:s0 + h0 + hg],
                                in_=o_sb[:, h0 * 2 * OW:(h0 + hg) * 2 * OW]
                                    .rearrange("p (g r j) -> p g r j",
                                               g=hg, r=2))
                elif p0 + 2 == s0 + g_store:
                    nc.sync.dma_start(
                        out=out_d[:, s0:s0 + g_store],
                        in_=o_sb.rearrange("p (g r j) -> p g r j",
                                           g=g_store, r=2))
                    del osb_tiles[s0]

            prev = None
            for p0 in range(0, n_img, 2):
                cur = (p0, stage1(p0))
                if prev is not None:
                    stage2(*prev)
                prev = cur
            stage2(*prev)
    nc.finalize()
    return nc


def _build_nc_fp32(n_img, n_terms, g_load=8, g_store=2):
    """general fp32 path, rank n_terms."""
    nc = bacc.Bacc("TRN2", target_bir_lowering=False)
    f32 = mybir.dt.float32
    R = n_terms
    x_d = nc.dram_tensor("x", (n_img, H, W), f32, kind="ExternalInput")
    at_d = nc.dram_tensor("at", (R, H, OH), f32, kind="ExternalInput")
    bt_d = nc.dram_tensor("bt", (R, W, OW), f32, kind="ExternalInput")
    out_d = nc.dram_tensor("out", (n_img, OH, OW), f32, kind="ExternalOutput")

    with TileContext(nc) as tc:
        with tc.tile_pool(name="consts", bufs=1) as cpool, \
             tc.tile_pool(name="data", bufs=4) as pool, \
             tc.tile_pool(name="psum", bufs=4, space="PSUM") as ppool:
            at = cpool.tile([H, R * OH], f32)
            nc.sync.dma_start(out=at.rearrange("p (r i) -> p r i", r=R), in_=at_d.rearrange("r h i -> h r i"))
            bt = cpool.tile([W, R * OW], f32)
            nc.sync.dma_start(out=bt.rearrange("p (r j) -> p r j", r=R), in_=bt_d.rearrange("r w j -> w r j"))

            for s0 in range(0, n_img, g_store):
                o_sb = pool.tile([128, g_store * 2 * OW], f32, tag="o_sb")
                for img in range(s0, s0 + g_store):
                    gi = img % g_load
                    if gi == 0:
                        g0 = img
                        xg = pool.tile([128, g_load * W], f32, tag="xg")
                        nc.sync.dma_start(
                            out=xg.rearrange("p (g w) -> p g w", g=g_load),
                            in_=x_d[g0:g0 + g_load].rearrange("g h w -> h g w"))
                    x_t = xg[:, gi * W:(gi + 1) * W]

                    s_p = ppool.tile([128, R * OH], f32, tag="s_p")
                    for r in range(R):
                        nc.tensor.matmul(s_p[:, r * OH:(r + 1) * OH], lhsT=x_t,
                                         rhs=at[:, r * OH:(r + 1) * OH],
                                         start=True, stop=True)
                    s_sb = pool.tile([128, R * OH], f32, tag="s_sb")
                    nc.scalar.copy(out=s_sb, in_=s_p)

                    oq = (img - s0) * 2 * OW
                    for blk in range(2):
                        o_p = ppool.tile([128, OW], f32, tag="o_p")
                        for r in range(R):
                            nc.tensor.matmul(
                                o_p,
                                lhsT=s_sb[:, r * OH + blk * 128: r * OH + (blk + 1) * 128],
                                rhs=bt[:, r * OW:(r + 1) * OW],
                                start=(r == 0), stop=(r == R - 1))
                        dst = o_sb[:, oq + blk * OW: oq + (blk + 1) * OW]
                        if blk == 0:
                            nc.vector.tensor_copy(dst, o_p)
                        else:
                            nc.scalar.copy(out=dst, in_=o_p)
                nc.sync.dma_start(
                    out=out_d[s0:s0 + g_store].rearrange("g (b p) j -> p g b j", b=2),
                    in_=o_sb.rearrange("p (g b j) -> p g b j", g=g_store, b=2))
    nc.finalize()
    return nc


_NC_CACHE = {}


def _get_nc(key, builder):
    if key not in _NC_CACHE:
        _NC_CACHE[key] = builder()
    return _NC_CACHE[key]


# ---------------------------------------------------------------- entry
def _run(x, kern, trace=False, n_cores=N_CORES):
    xf = np.ascontiguousarray(np.asarray(x, dtype=np.float32))
    k = np.asarray(kern, dtype=np.float32)
    b, c, h, w = xf.shape
    assert (h, w) == (H, W), (h, w)
    n_tot = b * c
    assert n_tot % n_cores == 0
    n_per = n_tot // n_cores
    imgs = xf.reshape(n_tot, h, w)

    terms = _factorize(k)

    if len(terms) == 1:
        A = _build_M(terms[0][0])
        Bm = _build_M(terms[0][1])
        nc = _get_nc(("f16", n_per), lambda: _build_nc_f16(n_per))
        x16 = imgs.astype(np.float16)
        # permute A rows to [even; odd] so MM2 block r produces rows 2p+r
        Ap = np.concatenate([A[0::2], A[1::2]], axis=0)
        at = np.ascontiguousarray(Ap.T.astype(np.float16))
        bt = np.ascontiguousarray(Bm.T.astype(np.float16))
        in_maps = [
            {"x": np.ascontiguousarray(
                 x16[i * n_per:(i + 1) * n_per].transpose(1, 0, 2)),
             "at": at, "bt": bt}
            for i in range(n_cores)
        ]
        res = run_bass_kernel_spmd(nc, in_maps, list(range(n_cores)), trace=trace)
        out = np.empty((n_tot, OH, OW), dtype=np.float32)
        for i in range(n_cores):
            o = res.results[i]["out"]  # (128, n_per, 2, OW) fp16
            # rows (2p+r) of image g live at o[p, g, r]; fuse gather + upcast
            out[i * n_per:(i + 1) * n_per] = (
                o.transpose(1, 0, 2, 3).reshape(n_per, OH, OW))
    else:
        R = len(terms)
        nc = _get_nc(("fp32", n_per, R), lambda: _build_nc_fp32(n_per, R))
        at = np.ascontiguousarray(
            np.stack([_build_M(u).T for (u, v) in terms]))
        bt = np.ascontiguousarray(
            np.stack([_build_M(v).T for (u, v) in terms]))
        in_maps = [
            {"x": imgs[i * n_per:(i + 1) * n_per], "at": at, "bt": bt}
            for i in range(n_cores)
        ]
        res = run_bass_kernel_spmd(nc, in_maps, list(range(n_cores)), trace=trace)
        out = np.concatenate([res.results[i]["out"] for i in range(n_cores)], axis=0)
    return out.reshape(b, c, OH, OW), res


def kernel(x, kernel):
    out, _ = _run(x, kernel, trace=False)
    return out
